# revision 19
# baseline (speedup 1.0000x reference)
"""Trainium2 Bass kernel for FFT-conv1d (= valid cross-correlation conv1d).

Reference computes, for x[N=64, C=64, W=4096], w[F=64, C=64, WW=16], b[F=64]:
    out[n, f, t] = sum_{c, j} x[n, c, t + j] * w[f, c, j] + b[f],  t in [0, 4081)

Strategy (v6, host-packed per-output-tile x windows + bf16 stores):
  - Data-parallel: shard N across 8 NeuronCores (8 samples per core);
    replicate w and b.
  - Direct convolution on the TensorEngine in bf16 (inputs cast on host;
    fp32 PSUM accumulation), K = 128 contraction = channels c (64) x 2
    adjacent taps: partitions 0-63 hold x[n,c,t], 64-127 hold x[n,c,t+1].
  - The host pre-packs x into xprep[pair, kt, 128, s, 528]: for each
    512-wide output tile kt, the 528-col window (with the +1-shifted copy
    on the upper partition half and zero tail padding) for both samples
    of a pair.  Each (pair, kt) is then ONE contiguous 270 KB
    128-partition DMA: per-output-tile dependency granularity (compute
    starts after ~270 KB instead of a whole sample pair), few large DMAs
    (HWDGE issue costs ~0.7 us each), no on-device shift or memset.
  - M = 64 output channels; the two samples of a pair go to PE column
    groups 0 / 64 (PSUM partitions 0-63 / 64-127), issued interleaved
    (jb outer, s inner) so both streams run concurrently on the array.
    8 matmuls (tap pairs) accumulate one PSUM bank [128, 512].
  - A few dummy matmuls on a zeroed tile right at kernel start warm the
    PE HAM clock gate while the first x tiles load.
  - Evacuation: one DVE tensor_scalar_add per bank (PSUM -> SBUF bf16,
    fused per-channel bias).  bf16 stores halve output DMA traffic; the
    host casts back to fp32.
"""

import numpy as np

N, C, W = 64, 64, 4096
F, WW = 64, 16
OUT_W = W - WW + 1  # 4081
N_CORES = 8
NPC = N // N_CORES  # samples per core = 8
NPAIR = NPC // 2  # sample pairs per core = 4
NKT = 8  # output tiles of 512 per sample
TW = 528  # x tile width: 2*7 + 512 = 526 cols used, padded to 528

_CACHE = {}


def _build_nc():
    from contextlib import ExitStack

    import concourse.bacc as bacc
    import concourse.mybir as mybir
    import concourse.tile as tile

    f32 = mybir.dt.float32
    bf16 = mybir.dt.bfloat16

    nc = bacc.Bacc(
        "TRN2", target_bir_lowering=False, debug=False, num_devices=N_CORES
    )
    xp_d = nc.dram_tensor(
        "xprep", [NPAIR, NKT, 128, 2, TW], bf16, kind="ExternalInput"
    ).ap()
    w_d = nc.dram_tensor("wstk", [128, 512], bf16, kind="ExternalInput").ap()
    b_d = nc.dram_tensor("bias2", [128, 1], f32, kind="ExternalInput").ap()
    o_d = nc.dram_tensor("out", [NPC, F, OUT_W], bf16, kind="ExternalOutput").ap()

    with tile.TileContext(nc) as tc:
        with ExitStack() as ctx:
            consts = ctx.enter_context(tc.tile_pool(name="consts", bufs=1))
            xpool = ctx.enter_context(tc.tile_pool(name="xs", bufs=16))
            opool = ctx.enter_context(tc.tile_pool(name="osb", bufs=3))
            pspool = ctx.enter_context(
                tc.tile_pool(name="ps", bufs=7, space="PSUM")
            )
            zpool = ctx.enter_context(
                tc.tile_pool(name="zps", bufs=1, space="PSUM")
            )

            # w/bias ride the gpsimd queue so the first sync/scalar-queue
            # DMAs are x data and compute starts as early as possible
            wsb = consts.tile([128, 512], bf16)
            nc.gpsimd.dma_start(out=wsb[:, :], in_=w_d[:, :])
            bsb = consts.tile([128, 1], f32)
            nc.gpsimd.dma_start(out=bsb[:, :], in_=b_d[:, :])

            # HAM warm-up: dummy matmuls on a zeroed tile keep the PE busy
            # while the first x tiles load, so the clock gate is at 8/8
            # (2.4 GHz) by the time the real matmul stream starts.
            zt = consts.tile([128, 512], bf16)
            nc.vector.memset(zt[:, :], 0.0)
            zps = zpool.tile([128, 512], f32)
            for _ in range(8):
                nc.tensor.matmul(
                    zps[0:64, :], lhsT=zt[:, 0:64], rhs=zt[:, :],
                    start=True, stop=True,
                )

            for pair in range(NPAIR):
                xts = []
                for kt in range(NKT):
                    xt = xpool.tile([128, 2, TW], bf16)
                    qeng = nc.sync if kt % 2 == 0 else nc.scalar
                    qeng.dma_start(out=xt[:, :, :], in_=xp_d[pair, kt])
                    xts.append(xt)

                o_pair = o_d[2 * pair : 2 * pair + 2].flatten_outer_dims()
                osb = opool.tile([128, NKT * 512], bf16)
                last_pair = pair == NPAIR - 1
                for kt in range(NKT):
                    nw = min(512, OUT_W - kt * 512)  # 497 on the last tile
                    ps = pspool.tile([128, 512], f32)
                    # sample s -> PE column group 64*s; jb outer / s inner
                    # so the two column-group streams interleave and run
                    # concurrently on disjoint array column halves.
                    for jb in range(8):
                        for s in range(2):
                            nc.tensor.matmul(
                                ps[64 * s : 64 * (s + 1), 0:nw],
                                lhsT=wsb[:, jb * 64 : (jb + 1) * 64],
                                rhs=xts[kt][:, s, 2 * jb : 2 * jb + nw],
                                start=(jb == 0),
                                stop=(jb == 7),
                            )
                    nc.vector.tensor_scalar_add(
                        osb[:, kt * 512 : kt * 512 + nw], ps[:, 0:nw], bsb[:, 0:1]
                    )
                    # ship finished output slices while later tiles compute;
                    # the last pair's stores go on the HWDGE queues (idle by
                    # then) so the gpsimd drain overlaps compute at the end
                    if not last_pair:
                        ocuts = {
                            3: [(0, 2048, nc.gpsimd)],
                            5: [(2048, 3072, nc.gpsimd)],
                            6: [(3072, 3584, nc.gpsimd)],
                            7: [(3584, OUT_W, nc.gpsimd)],
                        }
                    else:
                        ocuts = {
                            3: [(0, 2048, nc.scalar)],
                            5: [(2048, 3072, nc.sync)],
                            6: [(3072, 3584, nc.scalar)],
                            7: [(3584, 3968, nc.scalar), (3968, OUT_W, nc.sync)],
                        }
                    for lo, hi, eng in ocuts.get(kt, []):
                        eng.dma_start(
                            out=o_pair[:, lo:hi], in_=osb[:, lo:hi]
                        )

    nc.compile()
    return nc


def _get_nc():
    if "nc" not in _CACHE:
        _CACHE["nc"] = _build_nc()
    return _CACHE["nc"]


def _host_prep(w, b):
    import ml_dtypes

    # wstk[p*64 + c, jb*64 + f] = w[f, c, 2*jb + p]
    arr = np.ascontiguousarray(w, dtype=np.float32).reshape(F, C, 8, 2)
    wstk = np.ascontiguousarray(
        arr.transpose(3, 1, 2, 0).reshape(128, 512).astype(ml_dtypes.bfloat16)
    )
    bias2 = np.concatenate([b, b]).astype(np.float32).reshape(128, 1)
    bias2 = np.ascontiguousarray(bias2)
    return wstk, bias2


def _prep_x(x):
    """Pack x[N, C, W] (fp32) -> bf16 xprep[N//2, NKT, 128, 2, TW]:
    per output tile kt, the 528-col window; partitions 0-63 straight,
    64-127 shifted by +1; zero padding past the end of x."""
    import ml_dtypes

    xbf = x.astype(ml_dtypes.bfloat16)
    v = xbf.reshape(N // 2, 2, C, W)  # [pair, s, c, t]
    xp = np.zeros((N // 2, NKT, 128, 2, TW), dtype=ml_dtypes.bfloat16)
    for kt in range(NKT):
        base = kt * 512
        w0 = min(TW, W - base)
        w1 = min(TW, W - base - 1)
        # straight half: xp[p, kt, c, s, i] = x[2p+s, c, base+i]
        xp[:, kt, 0:64, :, 0:w0] = v[:, :, :, base : base + w0].transpose(
            0, 2, 1, 3
        )
        # shifted half: xp[p, kt, 64+c, s, i] = x[2p+s, c, base+1+i]
        xp[:, kt, 64:128, :, 0:w1] = v[
            :, :, :, base + 1 : base + 1 + w1
        ].transpose(0, 2, 1, 3)
    return np.ascontiguousarray(xp)


def _make_in_maps(x, w, b):
    wstk, bias2 = _host_prep(w, b)
    xp = _prep_x(x)
    ppc = NPAIR  # pairs per core
    return [
        {
            "xprep": np.ascontiguousarray(xp[i * ppc : (i + 1) * ppc]),
            "wstk": wstk,
            "bias2": bias2,
        }
        for i in range(N_CORES)
    ]


def kernel(x, w, b):
    from concourse.bass_utils import run_bass_kernel_spmd

    x = np.asarray(x, dtype=np.float32)
    w = np.asarray(w, dtype=np.float32)
    b = np.asarray(b, dtype=np.float32)
    assert x.shape == (N, C, W) and w.shape == (F, C, WW) and b.shape == (F,)

    nc = _get_nc()
    in_maps = _make_in_maps(x, w, b)
    res = run_bass_kernel_spmd(nc, in_maps, core_ids=list(range(N_CORES)))
    out = np.concatenate([r["out"] for r in res.results], axis=0)
    return out.astype(np.float32)


# revision 20
# speedup vs baseline: 1.1668x; 1.1668x over previous
"""Trainium2 Bass kernel for FFT-conv1d (= valid cross-correlation conv1d).

Reference computes, for x[N=64, C=64, W=4096], w[F=64, C=64, WW=16], b[F=64]:
    out[n, f, t] = sum_{c, j} x[n, c, t + j] * w[f, c, j] + b[f],  t in [0, 4081)

Strategy (v6, host-packed per-output-tile x windows + bf16 stores):
  - Data-parallel: shard N across 8 NeuronCores (8 samples per core);
    replicate w and b.
  - Direct convolution on the TensorEngine in bf16 (inputs cast on host;
    fp32 PSUM accumulation), K = 128 contraction = channels c (64) x 2
    adjacent taps: partitions 0-63 hold x[n,c,t], 64-127 hold x[n,c,t+1].
  - The host pre-packs x into xprep[pair, kt, 128, s, 528]: for each
    512-wide output tile kt, the 528-col window (with the +1-shifted copy
    on the upper partition half and zero tail padding) for both samples
    of a pair.  Each (pair, kt) is then ONE contiguous 270 KB
    128-partition DMA: per-output-tile dependency granularity (compute
    starts after ~270 KB instead of a whole sample pair), few large DMAs
    (HWDGE issue costs ~0.7 us each), no on-device shift or memset.
  - M = 64 output channels; the two samples of a pair go to PE column
    groups 0 / 64 (PSUM partitions 0-63 / 64-127), issued interleaved
    (jb outer, s inner) so both streams run concurrently on the array.
    8 matmuls (tap pairs) accumulate one PSUM bank [128, 512].
  - A few dummy matmuls on a zeroed tile right at kernel start warm the
    PE HAM clock gate while the first x tiles load.
  - Evacuation: one DVE tensor_scalar_add per bank (PSUM -> SBUF bf16,
    fused per-channel bias).  bf16 stores halve output DMA traffic; the
    host casts back to fp32.
"""

import numpy as np

N, C, W = 64, 64, 4096
F, WW = 64, 16
OUT_W = W - WW + 1  # 4081
N_CORES = 8
NPC = N // N_CORES  # samples per core = 8
NPAIR = NPC // 2  # sample pairs per core = 4
NKT = 8  # output tiles of 512 per sample
TW = 528  # x tile width: 2*7 + 512 = 526 cols used, padded to 528

_CACHE = {}


def _build_nc():
    from contextlib import ExitStack

    import concourse.bacc as bacc
    import concourse.mybir as mybir
    import concourse.tile as tile

    f32 = mybir.dt.float32
    bf16 = mybir.dt.bfloat16

    nc = bacc.Bacc(
        "TRN2", target_bir_lowering=False, debug=False, num_devices=N_CORES
    )
    xp_d = nc.dram_tensor(
        "xprep", [NPAIR, NKT, 128, 2, TW], bf16, kind="ExternalInput"
    ).ap()
    w_d = nc.dram_tensor("wstk", [128, 512], bf16, kind="ExternalInput").ap()
    b_d = nc.dram_tensor("bias2", [128, 1], f32, kind="ExternalInput").ap()
    o_d = nc.dram_tensor("out", [NPC, F, OUT_W], bf16, kind="ExternalOutput").ap()

    with tile.TileContext(nc) as tc:
        with ExitStack() as ctx:
            consts = ctx.enter_context(tc.tile_pool(name="consts", bufs=1))
            xpool = ctx.enter_context(tc.tile_pool(name="xs", bufs=16))
            opool = ctx.enter_context(tc.tile_pool(name="osb", bufs=3))
            pspool = ctx.enter_context(
                tc.tile_pool(name="ps", bufs=7, space="PSUM")
            )
            zpool = ctx.enter_context(
                tc.tile_pool(name="zps", bufs=1, space="PSUM")
            )

            # w/bias ride the gpsimd queue so the first sync/scalar-queue
            # DMAs are x data and compute starts as early as possible
            wsb = consts.tile([128, 512], bf16)
            nc.gpsimd.dma_start(out=wsb[:, :], in_=w_d[:, :])
            bsb = consts.tile([128, 1], f32)
            nc.gpsimd.dma_start(out=bsb[:, :], in_=b_d[:, :])

            # HAM warm-up: dummy matmuls on a zeroed tile keep the PE busy
            # while the first x tiles load, so the clock gate is at 8/8
            # (2.4 GHz) by the time the real matmul stream starts.
            zt = consts.tile([128, 512], bf16)
            nc.vector.memset(zt[:, :], 0.0)
            zps = zpool.tile([128, 512], f32)
            for _ in range(9):
                nc.tensor.matmul(
                    zps[0:64, :], lhsT=zt[:, 0:64], rhs=zt[:, :],
                    start=True, stop=True,
                )

            for pair in range(NPAIR):
                xts = []
                for kt in range(NKT):
                    xt = xpool.tile([128, 2, TW], bf16)
                    qeng = nc.sync if kt % 2 == 0 else nc.scalar
                    qeng.dma_start(out=xt[:, :, :], in_=xp_d[pair, kt])
                    xts.append(xt)

                o_pair = o_d[2 * pair : 2 * pair + 2].flatten_outer_dims()
                osb = opool.tile([128, NKT * 512], bf16)
                last_pair = pair == NPAIR - 1
                for kt in range(NKT):
                    ps = pspool.tile([128, 512], f32)
                    # sample s -> PE column group 64*s; jb outer / s inner
                    # so the two column-group streams interleave and run
                    # concurrently on disjoint array column halves.
                    for jb in range(8):
                        for s in range(2):
                            nc.tensor.matmul(
                                ps[64 * s : 64 * (s + 1), :],
                                lhsT=wsb[:, jb * 64 : (jb + 1) * 64],
                                rhs=xts[kt][:, s, 2 * jb : 2 * jb + 512],
                                start=(jb == 0),
                                stop=(jb == 7),
                            )
                    nc.vector.tensor_scalar_add(
                        osb[:, kt * 512 : (kt + 1) * 512], ps[:, :], bsb[:, 0:1]
                    )
                    # ship finished output slices while later tiles compute;
                    # the last pair's stores go on the HWDGE queues (idle by
                    # then) so the gpsimd drain overlaps compute at the end
                    if not last_pair:
                        ocuts = {
                            3: [(0, 2048, nc.gpsimd)],
                            5: [(2048, 3072, nc.gpsimd)],
                            6: [(3072, 3584, nc.gpsimd)],
                            7: [(3584, OUT_W, nc.gpsimd)],
                        }
                    else:
                        ocuts = {
                            3: [(0, 2048, nc.scalar)],
                            5: [(2048, 3072, nc.sync)],
                            6: [(3072, 3584, nc.scalar)],
                            7: [(3584, 3968, nc.scalar), (3968, OUT_W, nc.sync)],
                        }
                    for lo, hi, eng in ocuts.get(kt, []):
                        eng.dma_start(
                            out=o_pair[:, lo:hi], in_=osb[:, lo:hi]
                        )

    nc.compile()
    return nc


def _get_nc():
    if "nc" not in _CACHE:
        _CACHE["nc"] = _build_nc()
    return _CACHE["nc"]


def _host_prep(w, b):
    import ml_dtypes

    # wstk[p*64 + c, jb*64 + f] = w[f, c, 2*jb + p]
    arr = np.ascontiguousarray(w, dtype=np.float32).reshape(F, C, 8, 2)
    wstk = np.ascontiguousarray(
        arr.transpose(3, 1, 2, 0).reshape(128, 512).astype(ml_dtypes.bfloat16)
    )
    bias2 = np.concatenate([b, b]).astype(np.float32).reshape(128, 1)
    bias2 = np.ascontiguousarray(bias2)
    return wstk, bias2


def _prep_x(x):
    """Pack x[N, C, W] (fp32) -> bf16 xprep[N//2, NKT, 128, 2, TW]:
    per output tile kt, the 528-col window; partitions 0-63 straight,
    64-127 shifted by +1; zero padding past the end of x."""
    import ml_dtypes

    xbf = x.astype(ml_dtypes.bfloat16)
    v = xbf.reshape(N // 2, 2, C, W)  # [pair, s, c, t]
    xp = np.zeros((N // 2, NKT, 128, 2, TW), dtype=ml_dtypes.bfloat16)
    for kt in range(NKT):
        base = kt * 512
        w0 = min(TW, W - base)
        w1 = min(TW, W - base - 1)
        # straight half: xp[p, kt, c, s, i] = x[2p+s, c, base+i]
        xp[:, kt, 0:64, :, 0:w0] = v[:, :, :, base : base + w0].transpose(
            0, 2, 1, 3
        )
        # shifted half: xp[p, kt, 64+c, s, i] = x[2p+s, c, base+1+i]
        xp[:, kt, 64:128, :, 0:w1] = v[
            :, :, :, base + 1 : base + 1 + w1
        ].transpose(0, 2, 1, 3)
    return np.ascontiguousarray(xp)


def _make_in_maps(x, w, b):
    wstk, bias2 = _host_prep(w, b)
    xp = _prep_x(x)
    ppc = NPAIR  # pairs per core
    return [
        {
            "xprep": np.ascontiguousarray(xp[i * ppc : (i + 1) * ppc]),
            "wstk": wstk,
            "bias2": bias2,
        }
        for i in range(N_CORES)
    ]


def kernel(x, w, b):
    from concourse.bass_utils import run_bass_kernel_spmd

    x = np.asarray(x, dtype=np.float32)
    w = np.asarray(w, dtype=np.float32)
    b = np.asarray(b, dtype=np.float32)
    assert x.shape == (N, C, W) and w.shape == (F, C, WW) and b.shape == (F,)

    nc = _get_nc()
    in_maps = _make_in_maps(x, w, b)
    res = run_bass_kernel_spmd(nc, in_maps, core_ids=list(range(N_CORES)))
    out = np.concatenate([r["out"] for r in res.results], axis=0)
    return out.astype(np.float32)


# revision 22
# speedup vs baseline: 1.1742x; 1.0063x over previous
"""Trainium2 Bass kernel for FFT-conv1d (= valid cross-correlation conv1d).

Reference computes, for x[N=64, C=64, W=4096], w[F=64, C=64, WW=16], b[F=64]:
    out[n, f, t] = sum_{c, j} x[n, c, t + j] * w[f, c, j] + b[f],  t in [0, 4081)

Strategy (v6, host-packed per-output-tile x windows + bf16 stores):
  - Data-parallel: shard N across 8 NeuronCores (8 samples per core);
    replicate w and b.
  - Direct convolution on the TensorEngine in bf16 (inputs cast on host;
    fp32 PSUM accumulation), K = 128 contraction = channels c (64) x 2
    adjacent taps: partitions 0-63 hold x[n,c,t], 64-127 hold x[n,c,t+1].
  - The host pre-packs x into xprep[pair, kt, 128, s, 528]: for each
    512-wide output tile kt, the 528-col window (with the +1-shifted copy
    on the upper partition half and zero tail padding) for both samples
    of a pair.  Each (pair, kt) is then ONE contiguous 270 KB
    128-partition DMA: per-output-tile dependency granularity (compute
    starts after ~270 KB instead of a whole sample pair), few large DMAs
    (HWDGE issue costs ~0.7 us each), no on-device shift or memset.
  - M = 64 output channels; the two samples of a pair go to PE column
    groups 0 / 64 (PSUM partitions 0-63 / 64-127), issued interleaved
    (jb outer, s inner) so both streams run concurrently on the array.
    8 matmuls (tap pairs) accumulate one PSUM bank [128, 512].
  - A few dummy matmuls on a zeroed tile right at kernel start warm the
    PE HAM clock gate while the first x tiles load.
  - Evacuation: one DVE tensor_scalar_add per bank (PSUM -> SBUF bf16,
    fused per-channel bias).  bf16 stores halve output DMA traffic; the
    host casts back to fp32.
"""

import numpy as np

N, C, W = 64, 64, 4096
F, WW = 64, 16
OUT_W = W - WW + 1  # 4081
N_CORES = 8
NPC = N // N_CORES  # samples per core = 8
NPAIR = NPC // 2  # sample pairs per core = 4
NKT = 8  # output tiles of 512 per sample
TW = 528  # x tile width: 2*7 + 512 = 526 cols used, padded to 528

_CACHE = {}


def _build_nc():
    from contextlib import ExitStack

    import concourse.bacc as bacc
    import concourse.mybir as mybir
    import concourse.tile as tile

    f32 = mybir.dt.float32
    bf16 = mybir.dt.bfloat16

    nc = bacc.Bacc(
        "TRN2", target_bir_lowering=False, debug=False, num_devices=N_CORES
    )
    xp_d = nc.dram_tensor(
        "xprep", [NPAIR, NKT, 128, 2, TW], bf16, kind="ExternalInput"
    ).ap()
    w_d = nc.dram_tensor("wstk", [128, 512], bf16, kind="ExternalInput").ap()
    b_d = nc.dram_tensor("bias2", [128, 1], f32, kind="ExternalInput").ap()
    o_d = nc.dram_tensor("out", [NPC, F, OUT_W], bf16, kind="ExternalOutput").ap()

    with tile.TileContext(nc) as tc:
        with ExitStack() as ctx:
            consts = ctx.enter_context(tc.tile_pool(name="consts", bufs=1))
            xpool = ctx.enter_context(tc.tile_pool(name="xs", bufs=16))
            opool = ctx.enter_context(tc.tile_pool(name="osb", bufs=3))
            pspool = ctx.enter_context(
                tc.tile_pool(name="ps", bufs=7, space="PSUM")
            )
            zpool = ctx.enter_context(
                tc.tile_pool(name="zps", bufs=1, space="PSUM")
            )

            # w/bias ride the gpsimd queue so the first sync/scalar-queue
            # DMAs are x data and compute starts as early as possible
            wsb = consts.tile([128, 512], bf16)
            nc.gpsimd.dma_start(out=wsb[:, :], in_=w_d[:, :])
            bsb = consts.tile([128, 1], f32)
            nc.gpsimd.dma_start(out=bsb[:, :], in_=b_d[:, :])

            # HAM warm-up: dummy matmuls on a zeroed tile keep the PE busy
            # while the first x tiles load, so the clock gate is at 8/8
            # (2.4 GHz) by the time the real matmul stream starts.
            zt = consts.tile([128, 512], bf16)
            nc.vector.memset(zt[:, :], 0.0)
            zps = zpool.tile([128, 512], f32)
            for _ in range(9):
                nc.tensor.matmul(
                    zps[0:64, :], lhsT=zt[:, 0:64], rhs=zt[:, :],
                    start=True, stop=True,
                )

            for pair in range(NPAIR):
                xts = []
                for kt in range(NKT):
                    xt = xpool.tile([128, 2, TW], bf16)
                    qeng = nc.sync if kt % 2 == 0 else nc.scalar
                    qeng.dma_start(out=xt[:, :, :], in_=xp_d[pair, kt])
                    xts.append(xt)

                o_pair = o_d[2 * pair : 2 * pair + 2].flatten_outer_dims()
                osb = opool.tile([128, NKT * 512], bf16)
                last_pair = pair == NPAIR - 1
                for kt in range(NKT):
                    ps = pspool.tile([128, 512], f32)
                    # sample s -> PE column group 64*s; jb outer / s inner
                    # so the two column-group streams interleave and run
                    # concurrently on disjoint array column halves.
                    for jb in range(8):
                        for s in range(2):
                            nc.tensor.matmul(
                                ps[64 * s : 64 * (s + 1), :],
                                lhsT=wsb[:, jb * 64 : (jb + 1) * 64],
                                rhs=xts[kt][:, s, 2 * jb : 2 * jb + 512],
                                start=(jb == 0),
                                stop=(jb == 7),
                            )
                    if last_pair and kt == 7:
                        nc.vector.tensor_scalar_add(
                            osb[:, 3584:3840], ps[:, 0:256], bsb[:, 0:1]
                        )
                        nc.vector.tensor_scalar_add(
                            osb[:, 3840:4096], ps[:, 256:512], bsb[:, 0:1]
                        )
                    else:
                        nc.vector.tensor_scalar_add(
                            osb[:, kt * 512 : (kt + 1) * 512],
                            ps[:, :],
                            bsb[:, 0:1],
                        )
                    # ship finished output slices while later tiles compute;
                    # the last pair's stores go on the HWDGE queues (idle by
                    # then) so the gpsimd drain overlaps compute at the end
                    if not last_pair:
                        ocuts = {
                            3: [(0, 2048, nc.gpsimd)],
                            5: [(2048, 3072, nc.gpsimd)],
                            6: [(3072, 3584, nc.gpsimd)],
                            7: [(3584, OUT_W, nc.gpsimd)],
                        }
                    else:
                        ocuts = {
                            3: [(0, 2048, nc.scalar)],
                            5: [(2048, 3072, nc.sync)],
                            6: [(3072, 3584, nc.scalar)],
                            7: [(3584, 3840, nc.scalar), (3840, OUT_W, nc.sync)],
                        }
                    for lo, hi, eng in ocuts.get(kt, []):
                        eng.dma_start(
                            out=o_pair[:, lo:hi], in_=osb[:, lo:hi]
                        )

    nc.compile()
    return nc


def _get_nc():
    if "nc" not in _CACHE:
        _CACHE["nc"] = _build_nc()
    return _CACHE["nc"]


def _host_prep(w, b):
    import ml_dtypes

    # wstk[p*64 + c, jb*64 + f] = w[f, c, 2*jb + p]
    arr = np.ascontiguousarray(w, dtype=np.float32).reshape(F, C, 8, 2)
    wstk = np.ascontiguousarray(
        arr.transpose(3, 1, 2, 0).reshape(128, 512).astype(ml_dtypes.bfloat16)
    )
    bias2 = np.concatenate([b, b]).astype(np.float32).reshape(128, 1)
    bias2 = np.ascontiguousarray(bias2)
    return wstk, bias2


def _prep_x(x):
    """Pack x[N, C, W] (fp32) -> bf16 xprep[N//2, NKT, 128, 2, TW]:
    per output tile kt, the 528-col window; partitions 0-63 straight,
    64-127 shifted by +1; zero padding past the end of x."""
    import ml_dtypes

    xbf = x.astype(ml_dtypes.bfloat16)
    v = xbf.reshape(N // 2, 2, C, W)  # [pair, s, c, t]
    xp = np.zeros((N // 2, NKT, 128, 2, TW), dtype=ml_dtypes.bfloat16)
    for kt in range(NKT):
        base = kt * 512
        w0 = min(TW, W - base)
        w1 = min(TW, W - base - 1)
        # straight half: xp[p, kt, c, s, i] = x[2p+s, c, base+i]
        xp[:, kt, 0:64, :, 0:w0] = v[:, :, :, base : base + w0].transpose(
            0, 2, 1, 3
        )
        # shifted half: xp[p, kt, 64+c, s, i] = x[2p+s, c, base+1+i]
        xp[:, kt, 64:128, :, 0:w1] = v[
            :, :, :, base + 1 : base + 1 + w1
        ].transpose(0, 2, 1, 3)
    return np.ascontiguousarray(xp)


def _make_in_maps(x, w, b):
    wstk, bias2 = _host_prep(w, b)
    xp = _prep_x(x)
    ppc = NPAIR  # pairs per core
    return [
        {
            "xprep": np.ascontiguousarray(xp[i * ppc : (i + 1) * ppc]),
            "wstk": wstk,
            "bias2": bias2,
        }
        for i in range(N_CORES)
    ]


def kernel(x, w, b):
    from concourse.bass_utils import run_bass_kernel_spmd

    x = np.asarray(x, dtype=np.float32)
    w = np.asarray(w, dtype=np.float32)
    b = np.asarray(b, dtype=np.float32)
    assert x.shape == (N, C, W) and w.shape == (F, C, WW) and b.shape == (F,)

    nc = _get_nc()
    in_maps = _make_in_maps(x, w, b)
    res = run_bass_kernel_spmd(nc, in_maps, core_ids=list(range(N_CORES)))
    out = np.concatenate([r["out"] for r in res.results], axis=0)
    return out.astype(np.float32)
